# revision 10
# baseline (speedup 1.0000x reference)
"""Multi-head attention (B=2, S=2048, H=1024, NH=16, HD=64) on 8 trn2 cores.

Sharding: tensor-parallel over heads. Core c owns heads {2c, 2c+1}, i.e.
feature columns [128c, 128c+128) of q/k/v. Wq/Wk/Wv are column-sharded,
Wo row-sharded; each core computes a full-shape partial output and the
host sums the 8 partials (the row-parallel reduce) during unshard.

On-chip layout is feature-major ("transposed"): the host passes
hsT = hidden_states.T so both matmul operands of every projection have
the contraction dim on partitions. Attention works on scoresT[tk, tq];
softmax's normalizer comes from a ones-column augmented V matmul.

v2 schedule: QKV chains, attention p-blocks, next-batch chains and
prev-batch output-projection blocks are interleaved in emission order so
the ACT engine (exp, the throughput floor) runs continuously from ~10us
in. Normalizer: reciprocal_approx_fast (DVE) + partition_broadcast
(GpSimd) instead of a 4us 1-partition RECIPROCAL + DRAM roundtrip.
Head-B context is accumulated on partitions 64:128 directly (ones row at
stationary col 0) so no SBUF->SBUF partition-moving DMA is needed.
"""

import numpy as np

B, S, H, NH, HD = 2, 2048, 1024, 16, 64
NCORES = 8
JC = 128  # head-columns per core (2 heads x 64)
T = B * S  # 4096 tokens
TQB = 512  # tq block
NKT = S // 128  # 16 tk blocks per batch
NCH = S // TQB  # 4 token chunks per batch
BASE = 10000.0

_nc_cache = [None]

_LDW_OPT = False


def _patch_ldw_opt():
    from concourse import bass_utils as _bu

    if getattr(_bu, "_ldw_patched", False):
        return
    _orig = _bu.run_command

    def _patched(argv, **kw):
        argv = [
            a.replace("--enable-ldw-opt=false", "--enable-ldw-opt=true")
            if _LDW_OPT and isinstance(a, str)
            else a
            for a in argv
        ]
        return _orig(argv, **kw)

    _bu.run_command = _patched
    _bu._ldw_patched = True


def _build():
    _patch_ldw_opt()
    import concourse.tile as tile
    from concourse import bacc, mybir
    from concourse.masks import make_identity

    F32 = mybir.dt.float32
    F32R = mybir.dt.float32r
    F16 = mybir.dt.float16
    EXP = mybir.ActivationFunctionType.Exp

    nc = bacc.Bacc("TRN2", target_bir_lowering=False, debug=False)

    hsT = nc.dram_tensor("hsT", [H, T], F32R, kind="ExternalInput").ap()
    wqT = nc.dram_tensor("wqT", [H, JC], F32R, kind="ExternalInput").ap()
    wkT = nc.dram_tensor("wkT", [H, JC], F32R, kind="ExternalInput").ap()
    wvT = nc.dram_tensor("wvT", [H, JC], F32R, kind="ExternalInput").ap()
    woJI = nc.dram_tensor("woJI", [JC, H], F32R, kind="ExternalInput").ap()
    cosT = nc.dram_tensor("cosT", [128, S], F32, kind="ExternalInput").ap()
    sinTs = nc.dram_tensor("sinTs", [128, S], F32, kind="ExternalInput").ap()
    out = nc.dram_tensor("out", [T, H], F32, kind="ExternalOutput").ap()

    with tile.TileContext(nc) as tc:
        with (
            tc.tile_pool(name="wts", bufs=1) as wts,
            tc.tile_pool(name="tabs", bufs=1) as tabs,
            tc.tile_pool(name="hst", bufs=3) as hst,
            tc.tile_pool(name="qkv", bufs=2) as qkvp,
            tc.tile_pool(name="rope", bufs=2) as ropep,
            tc.tile_pool(name="vaug", bufs=2) as vaugp,
            tc.tile_pool(name="expt", bufs=4) as exptp,
            tc.tile_pool(name="ctx", bufs=2) as ctxp,
            tc.tile_pool(name="nrm", bufs=2) as nrmp,
            tc.tile_pool(name="outs", bufs=4) as outsp,
            tc.tile_pool(name="scp", bufs=2, space="PSUM") as scp,
            tc.tile_pool(name="cxp", bufs=2, space="PSUM") as cxp,
            tc.tile_pool(name="ptp", bufs=2, space="PSUM") as ptp,
            tc.tile_pool(name="zdr", bufs=4, space="DRAM") as zdrp,
        ):
            # ---- persistent weights / tables (issued first; DMA prefetch) ----
            wq_sb = wts.tile([128, 8, JC], F32R, tag="wq")
            nc.sync.dma_start(
                out=wq_sb[:], in_=wqT[:, :].rearrange("(k p) j -> p k j", p=128)
            )
            wk_sb = wts.tile([128, 8, JC], F32R, tag="wk")
            nc.sync.dma_start(
                out=wk_sb[:], in_=wkT[:, :].rearrange("(k p) j -> p k j", p=128)
            )
            wv_sb = wts.tile([128, 8, JC], F32R, tag="wv")
            nc.sync.dma_start(
                out=wv_sb[:], in_=wvT[:, :].rearrange("(k p) j -> p k j", p=128)
            )
            wJ = wts.tile([128, H], F32R, tag="wj")
            nc.sync.dma_start(out=wJ[:], in_=woJI[:, :])
            cos_sb = tabs.tile([128, S], F32, tag="cos")
            nc.sync.dma_start(out=cos_sb[:], in_=cosT[:, :])
            sin_sb = tabs.tile([128, S], F32, tag="sin")
            nc.sync.dma_start(out=sin_sb[:], in_=sinTs[:, :])
            ident = tabs.tile([128, 128], F32, tag="ident")
            make_identity(nc, ident[:])
            onesc = tabs.tile([128, NKT], F32, tag="ones")
            nc.vector.memset(onesc[:], 1.0)

            # ---- per-batch state, created lazily ----
            st = {}

            def batch_state(b):
                if b in st:
                    return st[b]
                qT = qkvp.tile([128, S], F16, tag="qT", name=f"qT{b}")
                kT = qkvp.tile([128, S], F16, tag="kT", bufs=1, name=f"kT{b}")
                vT = qkvp.tile([128, S], F32, tag="vT", bufs=1, name=f"vT{b}")
                kZA = qkvp.tile([128, S], F16, tag="kZA", name=f"kZA{b}")
                nc.vector.memset(kZA[64:128, :], 0.0)
                kZB = qkvp.tile([128, S], F16, tag="kZB", name=f"kZB{b}")
                nc.vector.memset(kZB[0:64, :], 0.0)
                # augmented V: head A ctx dims at cols 0:64, ones col 64
                #              head B ctx dims at cols 64:128, ones col 0
                vA = vaugp.tile([128, NKT, 128], F16, tag="vA", name=f"vA{b}")
                nc.vector.memset(vA[:, :, 65:128], 0.0)
                nc.vector.tensor_copy(vA[:, :, 64], onesc[:])
                vB = vaugp.tile([128, NKT, 128], F16, tag="vB", name=f"vB{b}")
                nc.vector.memset(vB[:, :, 1:64], 0.0)
                nc.vector.tensor_copy(vB[:, :, 0], onesc[:])
                ctxS = ctxp.tile([128, S], F32R, tag="cts", name=f"ctxS{b}")
                st[b] = dict(qT=qT, kT=kT, vT=vT, kZA=kZA, kZB=kZB, vA=vA,
                             vB=vB, ctxS=ctxS)
                return st[b]

            chunk_cache = {}

            def chunk(b, nchi):
                if (b, nchi) not in chunk_cache:
                    t0 = b * S + nchi * TQB
                    c = hst.tile([128, 8, TQB], F32R, tag="hst")
                    nc.sync.dma_start(
                        out=c[:],
                        in_=hsT[:, t0 : t0 + TQB].rearrange(
                            "(k p) t -> p k t", p=128
                        ),
                    )
                    chunk_cache[(b, nchi)] = c
                return chunk_cache[(b, nchi)]

            def chain(b, kind, nchi):
                sb = batch_state(b)
                c = chunk(b, nchi)
                sl = slice(nchi * TQB, (nchi + 1) * TQB)
                w_sb = {"q": wq_sb, "k": wk_sb, "v": wv_sb}[kind]
                pt = ptp.tile([128, TQB], F32, tag="pt")
                for k in range(8):
                    nc.tensor.matmul(
                        pt[:], w_sb[:, k, :], c[:, k, :],
                        start=(k == 0), stop=(k == 7),
                    )
                if kind == "v":
                    nc.vector.tensor_copy(sb["vT"][:, sl], pt[:])
                    for tkl in range(NCH):
                        tkb = NCH * nchi + tkl
                        pt2 = ptp.tile([128, TQB], F32, tag="pt")
                        nc.tensor.transpose(
                            pt2[:, 0:128],
                            sb["vT"][:, 128 * tkb : 128 * (tkb + 1)],
                            ident[:],
                        )
                        nc.vector.tensor_copy(
                            sb["vA"][:, tkb, 0:64], pt2[:, 0:64]
                        )
                        nc.vector.tensor_copy(
                            sb["vB"][:, tkb, 64:128], pt2[:, 64:128]
                        )
                    return
                # q/k: RoPE
                raw = ropep.tile([128, TQB], F32, tag="raw")
                nc.vector.tensor_copy(raw[:], pt[:])
                rot = ropep.tile([128, TQB], F32, tag="rot")
                for h0 in (0, 64):
                    nc.sync.dma_start(
                        out=rot[h0 : h0 + 32, :], in_=raw[h0 + 32 : h0 + 64, :]
                    )
                    nc.sync.dma_start(
                        out=rot[h0 + 32 : h0 + 64, :], in_=raw[h0 : h0 + 32, :]
                    )
                t1 = ropep.tile([128, TQB], F32, tag="t1")
                nc.vector.tensor_mul(t1[:], raw[:], cos_sb[:, sl])
                t2 = ropep.tile([128, TQB], F32, tag="t2")
                nc.vector.tensor_mul(t2[:], rot[:], sin_sb[:, sl])
                dstT = sb["qT"] if kind == "q" else sb["kT"]
                nc.vector.tensor_add(dstT[:, sl], t1[:], t2[:])
                if kind == "k":
                    nc.vector.tensor_copy(
                        sb["kZA"][0:64, sl], dstT[0:64, sl]
                    )
                    nc.vector.tensor_copy(
                        sb["kZB"][64:128, sl], dstT[64:128, sl]
                    )

            cx_live = {}

            def attn_p(b, tqb, p):
                sb = batch_state(b)
                qsl = slice(tqb * TQB, (tqb + 1) * TQB)
                if p == 0:
                    cx_live[(b, "A")] = cxp.tile([128, TQB], F32, tag="cx",
                                                 name=f"cxA{b}_{tqb}")
                    cx_live[(b, "B")] = cxp.tile([128, TQB], F32, tag="cx",
                                                 name=f"cxB{b}_{tqb}")
                cxA = cx_live[(b, "A")]
                cxB = cx_live[(b, "B")]
                scA = scp.tile([128, 2 * TQB], F32, tag="sc")
                scB = scp.tile([128, 2 * TQB], F32, tag="sc")
                for t in range(2):
                    tkb = 2 * p + t
                    ksl = slice(128 * tkb, 128 * (tkb + 1))
                    nc.tensor.matmul(
                        scA[:, t * TQB : (t + 1) * TQB],
                        sb["kZA"][:, ksl], sb["qT"][:, qsl],
                        start=True, stop=True,
                    )
                    nc.tensor.matmul(
                        scB[:, t * TQB : (t + 1) * TQB],
                        sb["kZB"][:, ksl], sb["qT"][:, qsl],
                        start=True, stop=True,
                    )
                etA = exptp.tile([128, 2 * TQB], F16, tag="et")
                nc.scalar.activation(etA[:], scA[:], EXP, scale=0.125)
                etB = exptp.tile([128, 2 * TQB], F16, tag="et")
                nc.scalar.activation(etB[:], scB[:], EXP, scale=0.125)
                for t in range(2):
                    tkb = 2 * p + t
                    stt, spp = tkb == 0, tkb == NKT - 1
                    tsl = slice(t * TQB, (t + 1) * TQB)
                    nc.tensor.matmul(
                        cxA[:, :], sb["vA"][:, tkb, :], etA[:, tsl],
                        start=stt, stop=spp,
                    )
                    nc.tensor.matmul(
                        cxB[:, :], sb["vB"][:, tkb, :], etB[:, tsl],
                        start=stt, stop=spp,
                    )

            def norm_half(src_row, dst, cx_rows):
                zrow = nrmp.tile([1, TQB], F32, tag="zrow")
                nc.vector.tensor_copy(zrow[:], src_row)
                rz = nrmp.tile([1, TQB], F32, tag="rz")
                nc.vector.reciprocal_approx_fast(out=rz[:], in_=zrow[:])
                zd = zdrp.tile([1, TQB], F32, tag="zd")
                nc.sync.dma_start(out=zd[:], in_=rz[:])
                zr = nrmp.tile([64, TQB], F32, tag="zr")
                nc.sync.dma_start(
                    out=zr[:], in_=zd[0:1, :].to_broadcast([64, TQB])
                )
                nc.vector.tensor_mul(dst, cx_rows, zr[:])

            def normalize(b, tqb):
                sb = batch_state(b)
                qsl = slice(tqb * TQB, (tqb + 1) * TQB)
                cxA = cx_live.pop((b, "A"))
                cxB = cx_live.pop((b, "B"))
                norm_half(cxA[64:65, :], sb["ctxS"][0:64, qsl], cxA[0:64, :])
                norm_half(cxB[0:1, :], sb["ctxS"][64:128, qsl], cxB[64:128, :])

            def outproj(b, tq8):
                sb = batch_state(b)
                csl = slice(128 * tq8, 128 * (tq8 + 1))
                for ich in range(2):
                    isl = slice(ich * 512, (ich + 1) * 512)
                    po = ptp.tile([128, TQB], F32, tag="pt")
                    nc.tensor.matmul(
                        po[:], sb["ctxS"][:, csl], wJ[:, isl],
                        start=True, stop=True,
                    )
                    ot = outsp.tile([128, TQB], F32, tag="ot")
                    nc.vector.tensor_copy(ot[:], po[:])
                    nc.sync.dma_start(
                        out=out[
                            b * S + 128 * tq8 : b * S + 128 * (tq8 + 1), isl
                        ],
                        in_=ot[:],
                    )

            # ---- master schedule ----
            # fill queues: work to interleave into attention p-blocks
            def injections_for_batch(b):
                # emitted at odd p of tqb 1..3 of batch b (12 slots):
                # next batch's 12 chains
                inj = []
                if b + 1 < B:
                    for nchi in range(NCH):
                        for kind in ("v", "k", "q"):
                            inj.append(
                                lambda b2=b + 1, kk=kind, nn=nchi: chain(
                                    b2, kk, nn
                                )
                            )
                else:
                    # last batch: previous batch's 16 outproj blocks go in
                    # all 4 tqb (4 slots each)
                    pass
                return inj

            for b in range(B):
                batch_state(b)
                # tqb0 with chains of this batch interleaved (first batch)
                # or already emitted (later batches)
                if b == 0:
                    for nchi in range(NCH):
                        chain(b, "v", nchi)
                        chain(b, "k", nchi)
                        chain(b, "q", nchi)
                        attn_p(b, 0, 2 * nchi)
                        attn_p(b, 0, 2 * nchi + 1)
                else:
                    for p in range(8):
                        attn_p(b, 0, p)
                        if p % 2 == 1:
                            outproj(b - 1, (p - 1) // 2)
                normalize(b, 0)
                inj = injections_for_batch(b)
                ii = 0
                for tqb in range(1, NCH):
                    for p in range(8):
                        attn_p(b, tqb, p)
                        if p % 2 == 1:
                            if b + 1 < B:
                                if ii < len(inj):
                                    inj[ii]()
                                    ii += 1
                            else:
                                oq = 4 * (tqb - 1) + 4 + (p - 1) // 2
                                if oq < 16:
                                    outproj(b - 1, oq)
                    normalize(b, tqb)
                assert ii == len(inj)

            # tail: last batch's output projection
            for tq8 in range(16):
                outproj(B - 1, tq8)

    nc.compile()
    return nc


def _rope_tables():
    inv_freq = 1.0 / (BASE ** (np.arange(0, HD, 2, dtype=np.float64) / HD))
    t = np.arange(S, dtype=np.float64)
    freqs = np.outer(t, inv_freq)  # [S, 32]
    emb = np.concatenate([freqs, freqs], -1)  # [S, 64]
    cos = np.cos(emb).T.astype(np.float32)  # [64, S]
    sin = np.sin(emb).T.astype(np.float32)
    sin_signed = sin.copy()
    sin_signed[0:32] = -sin_signed[0:32]
    cosT = np.ascontiguousarray(np.tile(cos, (2, 1)))  # [128, S]
    sinTs = np.ascontiguousarray(np.tile(sin_signed, (2, 1)))
    return cosT, sinTs


def kernel(hidden_states, Wq, Wk, Wv, Wo):
    hidden_states = np.asarray(hidden_states, np.float32)
    Wq, Wk, Wv, Wo = (np.asarray(w, np.float32) for w in (Wq, Wk, Wv, Wo))

    if _nc_cache[0] is None:
        _nc_cache[0] = _build()
    nc = _nc_cache[0]

    hsT = np.ascontiguousarray(hidden_states.reshape(T, H).T)  # [H, T]
    cosT, sinTs = _rope_tables()
    in_maps = []
    for c in range(NCORES):
        sl = slice(JC * c, JC * (c + 1))
        in_maps.append(
            {
                "hsT": hsT,
                "wqT": np.ascontiguousarray(Wq[sl, :].T),
                "wkT": np.ascontiguousarray(Wk[sl, :].T),
                "wvT": np.ascontiguousarray(Wv[sl, :].T),
                "woJI": np.ascontiguousarray(Wo[:, sl].T),
                "cosT": cosT,
                "sinTs": sinTs,
            }
        )

    from concourse.bass_utils import run_bass_kernel_spmd

    res = run_bass_kernel_spmd(nc, in_maps, core_ids=list(range(NCORES)))
    acc = np.zeros((T, H), np.float64)
    for c in range(NCORES):
        acc += res.results[c]["out"]
    return acc.astype(np.float32).reshape(B, S, H)


# revision 25
# speedup vs baseline: 1.0330x; 1.0330x over previous
"""Multi-head attention (B=2, S=2048, H=1024, NH=16, HD=64) on 8 trn2 cores.

Sharding: tensor-parallel over heads. Core c owns heads {2c, 2c+1}, i.e.
feature columns [128c, 128c+128) of q/k/v. Wq/Wk/Wv are column-sharded,
Wo row-sharded; each core computes a full-shape partial output and the
host sums the 8 partials (the row-parallel reduce) during unshard.

On-chip layout is feature-major ("transposed"): the host passes
hsT = hidden_states.T so both matmul operands of every projection have
the contraction dim on partitions. Attention works on scoresT[tk, tq];
softmax's normalizer comes from a ones-column augmented V matmul.

Schedule: QKV chains, attention p-blocks, next-batch chains and
prev-batch output-projection blocks are interleaved in emission order so
the ACT engine (exp) and Tensor engine stay busy from ~10us in.
QKV inputs (hsT, Wq/Wk/Wv) are bf16 (half the DMA, 1 cyc/row matmuls,
cheap LDWEIGHTS); attention q/k/v/et are fp16 (full PE rate; fp8 was
measured numerically too lossy for the 2e-2 gate: peaked softmax rows
expose elementwise quantization directly). Out-projection fp32r.
Normalizer: reciprocal_approx_fast + DRAM-broadcast; head-B ctx
accumulates on partitions 64:128 directly (ones col 0 of augmented V).
"""

import numpy as np

B, S, H, NH, HD = 2, 2048, 1024, 16, 64
NCORES = 8
JC = 128  # head-columns per core (2 heads x 64)
T = B * S  # 4096 tokens
TQB = 512  # tq block
NKT = S // 128  # 16 tk blocks per batch
NCH = S // TQB  # 4 token chunks per batch
BASE = 10000.0

_nc_cache = [None]

_LDW_OPT = False


def _patch_ldw_opt():
    from concourse import bass_utils as _bu

    if getattr(_bu, "_ldw_patched", False):
        return
    _orig = _bu.run_command

    def _patched(argv, **kw):
        argv = [
            a.replace("--enable-ldw-opt=false", "--enable-ldw-opt=true")
            if _LDW_OPT and isinstance(a, str)
            else a
            for a in argv
        ]
        return _orig(argv, **kw)

    _bu.run_command = _patched
    _bu._ldw_patched = True


def _build():
    _patch_ldw_opt()
    import concourse.tile as tile
    from concourse import bacc, mybir
    from concourse.masks import make_identity

    F32 = mybir.dt.float32
    F32R = mybir.dt.float32r
    F16 = mybir.dt.float16
    BF16 = mybir.dt.bfloat16
    EXP = mybir.ActivationFunctionType.Exp

    nc = bacc.Bacc("TRN2", target_bir_lowering=False, debug=False)

    hsT = nc.dram_tensor("hsT", [H, T], BF16, kind="ExternalInput").ap()
    wqT = nc.dram_tensor("wqT", [H, JC], BF16, kind="ExternalInput").ap()
    wkT = nc.dram_tensor("wkT", [H, JC], BF16, kind="ExternalInput").ap()
    wvT = nc.dram_tensor("wvT", [H, JC], BF16, kind="ExternalInput").ap()
    woJI = nc.dram_tensor("woJI", [JC, H], F32R, kind="ExternalInput").ap()
    cosT = nc.dram_tensor("cosT", [128, S], F32, kind="ExternalInput").ap()
    sinTs = nc.dram_tensor("sinTs", [128, S], F32, kind="ExternalInput").ap()
    out = nc.dram_tensor("out", [T, H], F32, kind="ExternalOutput").ap()

    with tile.TileContext(nc) as tc:
        with (
            tc.tile_pool(name="wts", bufs=1) as wts,
            tc.tile_pool(name="tabs", bufs=1) as tabs,
            tc.tile_pool(name="hst", bufs=3) as hst,
            tc.tile_pool(name="qkv", bufs=2) as qkvp,
            tc.tile_pool(name="rope", bufs=2) as ropep,
            tc.tile_pool(name="vaug", bufs=2) as vaugp,
            tc.tile_pool(name="expt", bufs=4) as exptp,
            tc.tile_pool(name="ctx", bufs=2) as ctxp,
            tc.tile_pool(name="nrm", bufs=2) as nrmp,
            tc.tile_pool(name="outs", bufs=4) as outsp,
            tc.tile_pool(name="scp", bufs=2, space="PSUM") as scp,
            tc.tile_pool(name="cxp", bufs=2, space="PSUM") as cxp,
            tc.tile_pool(name="ptp", bufs=2, space="PSUM") as ptp,
            tc.tile_pool(name="zdr", bufs=4, space="DRAM") as zdrp,
        ):
            # ---- persistent weights / tables; wv/wk/wq + first chunk first
            # so the first chains start ASAP ----
            wv_sb = wts.tile([128, 8, JC], BF16, tag="wv")
            nc.sync.dma_start(
                out=wv_sb[:], in_=wvT[:, :].rearrange("(k p) j -> p k j", p=128)
            )
            wk_sb = wts.tile([128, 8, JC], BF16, tag="wk")
            nc.sync.dma_start(
                out=wk_sb[:], in_=wkT[:, :].rearrange("(k p) j -> p k j", p=128)
            )
            wq_sb = wts.tile([128, 8, JC], BF16, tag="wq")
            nc.sync.dma_start(
                out=wq_sb[:], in_=wqT[:, :].rearrange("(k p) j -> p k j", p=128)
            )
            cos_sb = tabs.tile([128, S], F32, tag="cos")
            nc.sync.dma_start(out=cos_sb[:], in_=cosT[:, :])
            sin_sb = tabs.tile([128, S], F32, tag="sin")
            nc.sync.dma_start(out=sin_sb[:], in_=sinTs[:, :])
            wJ = wts.tile([128, H], F32R, tag="wj")
            nc.sync.dma_start(out=wJ[:], in_=woJI[:, :])
            ident = tabs.tile([128, 128], F32, tag="ident")
            make_identity(nc, ident[:])
            onesc = tabs.tile([128, NKT], F32, tag="ones")
            nc.vector.memset(onesc[:], 1.0)

            # ---- per-batch state, created lazily ----
            st = {}

            def batch_state(b):
                if b in st:
                    return st[b]
                qT = qkvp.tile([128, S], F16, tag="qT", name=f"qT{b}")
                kT = qkvp.tile([128, S], F16, tag="kT", bufs=1, name=f"kT{b}")
                vT = qkvp.tile([128, S], F32, tag="vT", bufs=1, name=f"vT{b}")
                kZA = qkvp.tile([128, S], F16, tag="kZA", name=f"kZA{b}")
                nc.vector.memset(kZA[64:128, :], 0.0)
                kZB = qkvp.tile([128, S], F16, tag="kZB", name=f"kZB{b}")
                nc.vector.memset(kZB[0:64, :], 0.0)
                # augmented V: head A ctx dims at cols 0:64, ones col 64
                #              head B ctx dims at cols 64:128, ones col 0
                vA = vaugp.tile([128, NKT, 128], F16, tag="vA", name=f"vA{b}")
                nc.vector.memset(vA[:, :, 65:128], 0.0)
                nc.vector.tensor_copy(vA[:, :, 64], onesc[:])
                vB = vaugp.tile([128, NKT, 128], F16, tag="vB", name=f"vB{b}")
                nc.vector.memset(vB[:, :, 1:64], 0.0)
                nc.vector.tensor_copy(vB[:, :, 0], onesc[:])
                ctxS = ctxp.tile([128, S], F32R, tag="cts", name=f"ctxS{b}")
                st[b] = dict(qT=qT, kT=kT, vT=vT, kZA=kZA, kZB=kZB, vA=vA,
                             vB=vB, ctxS=ctxS)
                return st[b]

            chunk_cache = {}

            def chunk(b, nchi):
                if (b, nchi) not in chunk_cache:
                    t0 = b * S + nchi * TQB
                    c = hst.tile([128, 8, TQB], BF16, tag="hst")
                    nc.sync.dma_start(
                        out=c[:],
                        in_=hsT[:, t0 : t0 + TQB].rearrange(
                            "(k p) t -> p k t", p=128
                        ),
                    )
                    chunk_cache[(b, nchi)] = c
                return chunk_cache[(b, nchi)]

            def chain(b, kind, nchi):
                sb = batch_state(b)
                c = chunk(b, nchi)
                sl = slice(nchi * TQB, (nchi + 1) * TQB)
                w_sb = {"q": wq_sb, "k": wk_sb, "v": wv_sb}[kind]
                pt = ptp.tile([128, TQB], F32, tag="pt")
                for k in range(8):
                    nc.tensor.matmul(
                        pt[:], w_sb[:, k, :], c[:, k, :],
                        start=(k == 0), stop=(k == 7),
                    )
                if kind == "v":
                    nc.vector.tensor_copy(sb["vT"][:, sl], pt[:])
                    for tkl in range(NCH):
                        tkb = NCH * nchi + tkl
                        pt2 = ptp.tile([128, TQB], F32, tag="pt")
                        nc.tensor.transpose(
                            pt2[:, 0:128],
                            sb["vT"][:, 128 * tkb : 128 * (tkb + 1)],
                            ident[:],
                        )
                        nc.vector.tensor_copy(
                            sb["vA"][:, tkb, 0:64], pt2[:, 0:64]
                        )
                        nc.vector.tensor_copy(
                            sb["vB"][:, tkb, 64:128], pt2[:, 64:128]
                        )
                    return
                # q/k: RoPE
                raw = ropep.tile([128, TQB], F32, tag="raw")
                nc.vector.tensor_copy(raw[:], pt[:])
                rot = ropep.tile([128, TQB], F32, tag="rot")
                for h0 in (0, 64):
                    nc.sync.dma_start(
                        out=rot[h0 : h0 + 32, :], in_=raw[h0 + 32 : h0 + 64, :]
                    )
                    nc.sync.dma_start(
                        out=rot[h0 + 32 : h0 + 64, :], in_=raw[h0 : h0 + 32, :]
                    )
                t1 = ropep.tile([128, TQB], F32, tag="t1")
                nc.vector.tensor_mul(t1[:], raw[:], cos_sb[:, sl])
                t2 = ropep.tile([128, TQB], F32, tag="t2")
                nc.vector.tensor_mul(t2[:], rot[:], sin_sb[:, sl])
                dstT = sb["qT"] if kind == "q" else sb["kT"]
                nc.vector.tensor_add(dstT[:, sl], t1[:], t2[:])
                if kind == "k":
                    nc.vector.tensor_copy(
                        sb["kZA"][0:64, sl], dstT[0:64, sl]
                    )
                    nc.vector.tensor_copy(
                        sb["kZB"][64:128, sl], dstT[64:128, sl]
                    )

            cx_live = {}

            def attn_p(b, tqb, p):
                sb = batch_state(b)
                qsl = slice(tqb * TQB, (tqb + 1) * TQB)
                if p == 0:
                    cx_live[(b, "A")] = cxp.tile([128, TQB], F32, tag="cx",
                                                 name=f"cxA{b}_{tqb}")
                    cx_live[(b, "B")] = cxp.tile([128, TQB], F32, tag="cx",
                                                 name=f"cxB{b}_{tqb}")
                cxA = cx_live[(b, "A")]
                cxB = cx_live[(b, "B")]
                scA = scp.tile([128, 2 * TQB], F32, tag="sc")
                scB = scp.tile([128, 2 * TQB], F32, tag="sc")
                for t in range(2):
                    tkb = 2 * p + t
                    ksl = slice(128 * tkb, 128 * (tkb + 1))
                    nc.tensor.matmul(
                        scA[:, t * TQB : (t + 1) * TQB],
                        sb["kZA"][:, ksl], sb["qT"][:, qsl],
                        start=True, stop=True,
                    )
                    nc.tensor.matmul(
                        scB[:, t * TQB : (t + 1) * TQB],
                        sb["kZB"][:, ksl], sb["qT"][:, qsl],
                        start=True, stop=True,
                    )
                etA = exptp.tile([128, 2 * TQB], F16, tag="et")
                nc.scalar.activation(etA[:], scA[:], EXP, scale=0.125)
                etB = exptp.tile([128, 2 * TQB], F16, tag="et")
                nc.scalar.activation(etB[:], scB[:], EXP, scale=0.125)
                for t in range(2):
                    tkb = 2 * p + t
                    stt, spp = tkb == 0, tkb == NKT - 1
                    tsl = slice(t * TQB, (t + 1) * TQB)
                    nc.tensor.matmul(
                        cxA[:, :], sb["vA"][:, tkb, :], etA[:, tsl],
                        start=stt, stop=spp,
                    )
                    nc.tensor.matmul(
                        cxB[:, :], sb["vB"][:, tkb, :], etB[:, tsl],
                        start=stt, stop=spp,
                    )

            def norm_half(src_row, dst, cx_rows):
                zrow = nrmp.tile([1, TQB], F32, tag="zrow")
                nc.vector.tensor_copy(zrow[:], src_row)
                rz = nrmp.tile([1, TQB], F32, tag="rz")
                nc.vector.reciprocal_approx_fast(out=rz[:], in_=zrow[:])
                zd = zdrp.tile([1, TQB], F32, tag="zd")
                nc.sync.dma_start(out=zd[:], in_=rz[:])
                zr = nrmp.tile([64, TQB], F32, tag="zr")
                nc.sync.dma_start(
                    out=zr[:], in_=zd[0:1, :].to_broadcast([64, TQB])
                )
                nc.vector.tensor_mul(dst, cx_rows, zr[:])

            def normalize(b, tqb):
                sb = batch_state(b)
                qsl = slice(tqb * TQB, (tqb + 1) * TQB)
                cxA = cx_live.pop((b, "A"))
                cxB = cx_live.pop((b, "B"))
                norm_half(cxA[64:65, :], sb["ctxS"][0:64, qsl], cxA[0:64, :])
                norm_half(cxB[0:1, :], sb["ctxS"][64:128, qsl], cxB[64:128, :])

            def outproj(b, tq8):
                sb = batch_state(b)
                csl = slice(128 * tq8, 128 * (tq8 + 1))
                for ich in range(2):
                    isl = slice(ich * 512, (ich + 1) * 512)
                    po = ptp.tile([128, TQB], F32, tag="pt")
                    nc.tensor.matmul(
                        po[:], sb["ctxS"][:, csl], wJ[:, isl],
                        start=True, stop=True,
                    )
                    ot = outsp.tile([128, TQB], F32, tag="ot")
                    nc.vector.tensor_copy(ot[:], po[:])
                    nc.sync.dma_start(
                        out=out[
                            b * S + 128 * tq8 : b * S + 128 * (tq8 + 1), isl
                        ],
                        in_=ot[:],
                    )

            # ---- master schedule ----
            for b in range(B):
                batch_state(b)
                if b == 0:
                    # tqb0 with this batch's chains interleaved
                    for nchi in range(NCH):
                        chain(b, "v", nchi)
                        chain(b, "k", nchi)
                        chain(b, "q", nchi)
                        attn_p(b, 0, 2 * nchi)
                        attn_p(b, 0, 2 * nchi + 1)
                else:
                    # chains already emitted during batch b-1; start draining
                    # the previous batch's output projection
                    for p in range(8):
                        attn_p(b, 0, p)
                        if p % 2 == 1:
                            outproj(b - 1, (p - 1) // 2)
                normalize(b, 0)
                # injection work for the p-odd slots of tqb 1..3:
                # non-final batch: the next batch's 12 chains (1 per slot).
                # final batch: remaining outproj blocks of batch b-1 and
                # this batch (2 per slot), gated on normalize readiness.
                if b + 1 < B:
                    inj = [
                        lambda b2=b + 1, kk=kind, nn=nchi: chain(b2, kk, nn)
                        for nchi in range(NCH)
                        for kind in ("v", "k", "q")
                    ]
                    per_slot = 1
                    ready = lambda i, tqb: True
                else:
                    inj = [
                        lambda bb=bo, t8=t8: outproj(bb, t8)
                        for bo, t8s in ((b - 1, range(4, 16)), (b, range(12)))
                        for t8 in t8s
                    ]
                    per_slot = 2
                    # entry i >= 12 is outproj(b, i-12): its tqb block
                    # (i-12)//4 must have been normalized (tqb' < tqb)
                    ready = lambda i, tqb: i < 12 or (i - 12) < 4 * tqb
                ii = 0
                for tqb in range(1, NCH):
                    for p in range(8):
                        attn_p(b, tqb, p)
                        if p % 2 == 1:
                            for _ in range(per_slot):
                                if ii < len(inj) and ready(ii, tqb):
                                    inj[ii]()
                                    ii += 1
                    normalize(b, tqb)
                while ii < len(inj):
                    inj[ii]()
                    ii += 1

            # tail: last batch's final tqb output projection
            for tq8 in range(12, 16):
                outproj(B - 1, tq8)

    nc.compile()
    return nc


def _rope_tables():
    inv_freq = 1.0 / (BASE ** (np.arange(0, HD, 2, dtype=np.float64) / HD))
    t = np.arange(S, dtype=np.float64)
    freqs = np.outer(t, inv_freq)  # [S, 32]
    emb = np.concatenate([freqs, freqs], -1)  # [S, 64]
    cos = np.cos(emb).T.astype(np.float32)  # [64, S]
    sin = np.sin(emb).T.astype(np.float32)
    sin_signed = sin.copy()
    sin_signed[0:32] = -sin_signed[0:32]
    cosT = np.ascontiguousarray(np.tile(cos, (2, 1)))  # [128, S]
    sinTs = np.ascontiguousarray(np.tile(sin_signed, (2, 1)))
    return cosT, sinTs


def kernel(hidden_states, Wq, Wk, Wv, Wo):
    import ml_dtypes

    BF = ml_dtypes.bfloat16
    hidden_states = np.asarray(hidden_states, np.float32)
    Wq, Wk, Wv, Wo = (np.asarray(w, np.float32) for w in (Wq, Wk, Wv, Wo))

    if _nc_cache[0] is None:
        _nc_cache[0] = _build()
    nc = _nc_cache[0]

    hsT = np.ascontiguousarray(hidden_states.reshape(T, H).T).astype(BF)
    cosT, sinTs = _rope_tables()
    in_maps = []
    for c in range(NCORES):
        sl = slice(JC * c, JC * (c + 1))
        in_maps.append(
            {
                "hsT": hsT,
                "wqT": np.ascontiguousarray(Wq[sl, :].T).astype(BF),
                "wkT": np.ascontiguousarray(Wk[sl, :].T).astype(BF),
                "wvT": np.ascontiguousarray(Wv[sl, :].T).astype(BF),
                "woJI": np.ascontiguousarray(Wo[:, sl].T),
                "cosT": cosT,
                "sinTs": sinTs,
            }
        )

    from concourse.bass_utils import run_bass_kernel_spmd

    res = run_bass_kernel_spmd(nc, in_maps, core_ids=list(range(NCORES)))
    acc = np.zeros((T, H), np.float64)
    for c in range(NCORES):
        acc += res.results[c]["out"]
    return acc.astype(np.float32).reshape(B, S, H)


# revision 32
# speedup vs baseline: 1.2471x; 1.2073x over previous
"""Multi-head attention (B=2, S=2048, H=1024, NH=16, HD=64) on 8 trn2 cores.

Sharding: tensor-parallel over heads. Core c owns heads {2c, 2c+1}, i.e.
feature columns [128c, 128c+128) of q/k/v. Wq/Wk/Wv are column-sharded,
Wo row-sharded; each core computes a full-shape partial output and the
host sums the 8 partials (the row-parallel reduce) during unshard.

On-chip layout is feature-major ("transposed"): the host passes
hsT = hidden_states.T so both matmul operands of every projection have
the contraction dim on partitions. Attention works on scoresT[tk, tq];
softmax's normalizer comes from a ones-column augmented V matmul.

Schedule: QKV chains, attention p-blocks, next-batch chains and
prev-batch output-projection blocks are interleaved in emission order so
the ACT engine (exp) and Tensor engine stay busy from ~10us in.
QKV inputs (hsT, Wq/Wk/Wv) are bf16 (half the DMA, 1 cyc/row matmuls,
cheap LDWEIGHTS); attention q/k/v/et are fp16 (full PE rate; fp8 was
measured numerically too lossy for the 2e-2 gate: peaked softmax rows
expose elementwise quantization directly). Out-projection fp32r.
Normalizer: reciprocal_approx_fast + DRAM-broadcast; head-B ctx
accumulates on partitions 64:128 directly (ones col 0 of augmented V).
"""

import numpy as np

B, S, H, NH, HD = 2, 2048, 1024, 16, 64
NCORES = 8
JC = 128  # head-columns per core (2 heads x 64)
T = B * S  # 4096 tokens
TQB = 512  # tq block
NKT = S // 128  # 16 tk blocks per batch
NCH = S // TQB  # 4 token chunks per batch
BASE = 10000.0

_nc_cache = [None]

_LDW_OPT = False


def _patch_ldw_opt():
    from concourse import bass_utils as _bu

    if getattr(_bu, "_ldw_patched", False):
        return
    _orig = _bu.run_command

    def _patched(argv, **kw):
        argv = [
            a.replace("--enable-ldw-opt=false", "--enable-ldw-opt=true")
            if _LDW_OPT and isinstance(a, str)
            else a
            for a in argv
        ]
        return _orig(argv, **kw)

    _bu.run_command = _patched
    _bu._ldw_patched = True


def _build():
    _patch_ldw_opt()
    import concourse.tile as tile
    from concourse import bacc, mybir
    from concourse.masks import make_identity

    F32 = mybir.dt.float32
    F32R = mybir.dt.float32r
    F16 = mybir.dt.float16
    BF16 = mybir.dt.bfloat16
    EXP = mybir.ActivationFunctionType.Exp

    nc = bacc.Bacc("TRN2", target_bir_lowering=False, debug=False)

    hsT = nc.dram_tensor("hsT", [H, T], BF16, kind="ExternalInput").ap()
    wqT = nc.dram_tensor("wqT", [H, JC], BF16, kind="ExternalInput").ap()
    wkT = nc.dram_tensor("wkT", [H, JC], BF16, kind="ExternalInput").ap()
    wvT = nc.dram_tensor("wvT", [H, JC], BF16, kind="ExternalInput").ap()
    woJI = nc.dram_tensor("woJI", [JC, H], F32R, kind="ExternalInput").ap()
    cosT = nc.dram_tensor("cosT", [128, S], F32, kind="ExternalInput").ap()
    sinTs = nc.dram_tensor("sinTs", [128, S], F32, kind="ExternalInput").ap()
    out = nc.dram_tensor("out", [T, H], F32, kind="ExternalOutput").ap()

    with tile.TileContext(nc) as tc:
        with (
            tc.tile_pool(name="wts", bufs=1) as wts,
            tc.tile_pool(name="tabs", bufs=1) as tabs,
            tc.tile_pool(name="hst", bufs=3) as hst,
            tc.tile_pool(name="qkv", bufs=2) as qkvp,
            tc.tile_pool(name="rope", bufs=2) as ropep,
            tc.tile_pool(name="vaug", bufs=2) as vaugp,
            tc.tile_pool(name="expt", bufs=4) as exptp,
            tc.tile_pool(name="ctx", bufs=2) as ctxp,
            tc.tile_pool(name="nrm", bufs=2) as nrmp,
            tc.tile_pool(name="outs", bufs=4) as outsp,
            tc.tile_pool(name="scp", bufs=2, space="PSUM") as scp,
            tc.tile_pool(name="cxp", bufs=2, space="PSUM") as cxp,
            tc.tile_pool(name="ptp", bufs=2, space="PSUM") as ptp,
            tc.tile_pool(name="zdr", bufs=4, space="DRAM") as zdrp,
        ):
            # ---- persistent weights / tables; wv/wk/wq + first chunk first
            # so the first chains start ASAP ----
            wv_sb = wts.tile([128, 8, JC], BF16, tag="wv")
            nc.sync.dma_start(
                out=wv_sb[:], in_=wvT[:, :].rearrange("(k p) j -> p k j", p=128)
            )
            wk_sb = wts.tile([128, 8, JC], BF16, tag="wk")
            nc.sync.dma_start(
                out=wk_sb[:], in_=wkT[:, :].rearrange("(k p) j -> p k j", p=128)
            )
            wq_sb = wts.tile([128, 8, JC], BF16, tag="wq")
            nc.sync.dma_start(
                out=wq_sb[:], in_=wqT[:, :].rearrange("(k p) j -> p k j", p=128)
            )

            # ---- per-batch state, created lazily ----
            st = {}

            def batch_state(b):
                if b in st:
                    return st[b]
                qT = qkvp.tile([128, S], F16, tag="qT", name=f"qT{b}")
                kT = qkvp.tile([128, S], F16, tag="kT", bufs=1, name=f"kT{b}")
                vT = qkvp.tile([128, S], F16, tag="vT", bufs=1, name=f"vT{b}")
                kZA = qkvp.tile([128, S], F16, tag="kZA", name=f"kZA{b}")
                nc.vector.memset(kZA[64:128, :], 0.0)
                kZB = qkvp.tile([128, S], F16, tag="kZB", name=f"kZB{b}")
                nc.vector.memset(kZB[0:64, :], 0.0)
                # augmented V: head A ctx dims at cols 0:64, ones col 64
                #              head B ctx dims at cols 64:128, ones col 0
                vA = vaugp.tile([128, NKT, 128], F16, tag="vA", name=f"vA{b}")
                nc.vector.memset(vA[:, :, 65:128], 0.0)
                nc.vector.tensor_copy(vA[:, :, 64], onesc[:])
                vB = vaugp.tile([128, NKT, 128], F16, tag="vB", name=f"vB{b}")
                nc.vector.memset(vB[:, :, 1:64], 0.0)
                nc.vector.tensor_copy(vB[:, :, 0], onesc[:])
                ctxS = ctxp.tile([128, S], F32R, tag="cts", name=f"ctxS{b}")
                st[b] = dict(qT=qT, kT=kT, vT=vT, kZA=kZA, kZB=kZB, vA=vA,
                             vB=vB, ctxS=ctxS)
                return st[b]

            chunk_cache = {}

            def chunk(b, nchi):
                if (b, nchi) not in chunk_cache:
                    t0 = b * S + nchi * TQB
                    c = hst.tile([128, 8, TQB], BF16, tag="hst")
                    nc.sync.dma_start(
                        out=c[:],
                        in_=hsT[:, t0 : t0 + TQB].rearrange(
                            "(k p) t -> p k t", p=128
                        ),
                    )
                    chunk_cache[(b, nchi)] = c
                return chunk_cache[(b, nchi)]

            # prefetch the first token chunks ahead of the big table DMAs
            chunk(0, 0)
            chunk(0, 1)
            cos_sb = tabs.tile([128, S], F32, tag="cos")
            nc.sync.dma_start(out=cos_sb[:], in_=cosT[:, :])
            sin_sb = tabs.tile([128, S], F32, tag="sin")
            nc.sync.dma_start(out=sin_sb[:], in_=sinTs[:, :])
            wJ = wts.tile([128, H], F32R, tag="wj")
            nc.sync.dma_start(out=wJ[:], in_=woJI[:, :])
            ident = tabs.tile([128, 128], F16, tag="ident")
            make_identity(nc, ident[:])
            onesc = tabs.tile([128, NKT], F32, tag="ones")
            nc.vector.memset(onesc[:], 1.0)

            def chain(b, kind, nchi):
                sb = batch_state(b)
                c = chunk(b, nchi)
                sl = slice(nchi * TQB, (nchi + 1) * TQB)
                w_sb = {"q": wq_sb, "k": wk_sb, "v": wv_sb}[kind]
                pt = ptp.tile([128, TQB], F32, tag="pt")
                for k in range(8):
                    nc.tensor.matmul(
                        pt[:], w_sb[:, k, :], c[:, k, :],
                        start=(k == 0), stop=(k == 7),
                    )
                if kind == "v":
                    nc.vector.tensor_copy(sb["vT"][:, sl], pt[:])
                    for tkl in range(NCH):
                        tkb = NCH * nchi + tkl
                        pt2 = ptp.tile([128, 2 * TQB], F16, tag="pt")
                        nc.tensor.transpose(
                            pt2[:, 0:128],
                            sb["vT"][:, 128 * tkb : 128 * (tkb + 1)],
                            ident[:],
                        )
                        nc.vector.tensor_copy(
                            sb["vA"][:, tkb, 0:64], pt2[:, 0:64]
                        )
                        nc.vector.tensor_copy(
                            sb["vB"][:, tkb, 64:128], pt2[:, 64:128]
                        )
                    return
                # q/k: RoPE
                raw = ropep.tile([128, TQB], F32, tag="raw")
                nc.vector.tensor_copy(raw[:], pt[:])
                rot = ropep.tile([128, TQB], F32, tag="rot")
                for h0 in (0, 64):
                    nc.sync.dma_start(
                        out=rot[h0 : h0 + 32, :], in_=raw[h0 + 32 : h0 + 64, :]
                    )
                    nc.sync.dma_start(
                        out=rot[h0 + 32 : h0 + 64, :], in_=raw[h0 : h0 + 32, :]
                    )
                t1 = ropep.tile([128, TQB], F32, tag="t1")
                nc.vector.tensor_mul(t1[:], raw[:], cos_sb[:, sl])
                t2 = ropep.tile([128, TQB], F32, tag="t2")
                nc.vector.tensor_mul(t2[:], rot[:], sin_sb[:, sl])
                dstT = sb["qT"] if kind == "q" else sb["kT"]
                nc.vector.tensor_add(dstT[:, sl], t1[:], t2[:])
                if kind == "k":
                    nc.vector.tensor_copy(
                        sb["kZA"][0:64, sl], dstT[0:64, sl]
                    )
                    nc.vector.tensor_copy(
                        sb["kZB"][64:128, sl], dstT[64:128, sl]
                    )

            cx_live = {}

            def attn_p(b, tqb, p):
                sb = batch_state(b)
                qsl = slice(tqb * TQB, (tqb + 1) * TQB)
                if p == 0:
                    cx_live[(b, "A")] = cxp.tile([128, TQB], F32, tag="cx",
                                                 name=f"cxA{b}_{tqb}")
                    cx_live[(b, "B")] = cxp.tile([128, TQB], F32, tag="cx",
                                                 name=f"cxB{b}_{tqb}")
                cxA = cx_live[(b, "A")]
                cxB = cx_live[(b, "B")]
                scA = scp.tile([128, 2 * TQB], F32, tag="sc")
                scB = scp.tile([128, 2 * TQB], F32, tag="sc")
                for t in range(2):
                    tkb = 2 * p + t
                    ksl = slice(128 * tkb, 128 * (tkb + 1))
                    nc.tensor.matmul(
                        scA[:, t * TQB : (t + 1) * TQB],
                        sb["kZA"][:, ksl], sb["qT"][:, qsl],
                        start=True, stop=True,
                    )
                    nc.tensor.matmul(
                        scB[:, t * TQB : (t + 1) * TQB],
                        sb["kZB"][:, ksl], sb["qT"][:, qsl],
                        start=True, stop=True,
                    )
                etA = exptp.tile([128, 2 * TQB], F16, tag="et")
                nc.scalar.activation(etA[:], scA[:], EXP, scale=0.125)
                etB = exptp.tile([128, 2 * TQB], F16, tag="et")
                nc.scalar.activation(etB[:], scB[:], EXP, scale=0.125)
                for t in range(2):
                    tkb = 2 * p + t
                    stt, spp = tkb == 0, tkb == NKT - 1
                    tsl = slice(t * TQB, (t + 1) * TQB)
                    nc.tensor.matmul(
                        cxA[:, :], sb["vA"][:, tkb, :], etA[:, tsl],
                        start=stt, stop=spp,
                    )
                    nc.tensor.matmul(
                        cxB[:, :], sb["vB"][:, tkb, :], etB[:, tsl],
                        start=stt, stop=spp,
                    )

            def norm_half(src_row, dst, cx_rows):
                zrow = nrmp.tile([1, TQB], F32, tag="zrow")
                nc.vector.tensor_copy(zrow[:], src_row)
                rz = nrmp.tile([1, TQB], F32, tag="rz")
                nc.vector.reciprocal_approx_fast(out=rz[:], in_=zrow[:])
                zd = zdrp.tile([1, TQB], F32, tag="zd")
                nc.sync.dma_start(out=zd[:], in_=rz[:])
                zr = nrmp.tile([64, TQB], F32, tag="zr")
                nc.sync.dma_start(
                    out=zr[:], in_=zd[0:1, :].to_broadcast([64, TQB])
                )
                nc.vector.tensor_mul(dst, cx_rows, zr[:])

            def normalize(b, tqb):
                sb = batch_state(b)
                qsl = slice(tqb * TQB, (tqb + 1) * TQB)
                cxA = cx_live.pop((b, "A"))
                cxB = cx_live.pop((b, "B"))
                norm_half(cxA[64:65, :], sb["ctxS"][0:64, qsl], cxA[0:64, :])
                norm_half(cxB[0:1, :], sb["ctxS"][64:128, qsl], cxB[64:128, :])

            def outproj(b, tq8):
                sb = batch_state(b)
                csl = slice(128 * tq8, 128 * (tq8 + 1))
                for ich in range(2):
                    isl = slice(ich * 512, (ich + 1) * 512)
                    po = ptp.tile([128, TQB], F32, tag="pt")
                    nc.tensor.matmul(
                        po[:], sb["ctxS"][:, csl], wJ[:, isl],
                        start=True, stop=True,
                    )
                    ot = outsp.tile([128, TQB], F32, tag="ot")
                    nc.vector.tensor_copy(ot[:], po[:])
                    nc.sync.dma_start(
                        out=out[
                            b * S + 128 * tq8 : b * S + 128 * (tq8 + 1), isl
                        ],
                        in_=ot[:],
                    )

            # ---- master schedule ----
            for b in range(B):
                batch_state(b)
                if b == 0:
                    # tqb0 with this batch's chains interleaved
                    for nchi in range(NCH):
                        chain(b, "v", nchi)
                        chain(b, "k", nchi)
                        chain(b, "q", nchi)
                        attn_p(b, 0, 2 * nchi)
                        attn_p(b, 0, 2 * nchi + 1)
                else:
                    # chains already emitted during batch b-1; start draining
                    # the previous batch's output projection
                    for p in range(8):
                        attn_p(b, 0, p)
                        if p % 2 == 1:
                            outproj(b - 1, (p - 1) // 2)
                normalize(b, 0)
                # injection work for the p-odd slots of tqb 1..3:
                # non-final batch: the next batch's 12 chains (1 per slot).
                # final batch: remaining outproj blocks of batch b-1 and
                # this batch (2 per slot), gated on normalize readiness.
                if b + 1 < B:
                    inj = [
                        lambda b2=b + 1, kk=kind, nn=nchi: chain(b2, kk, nn)
                        for nchi in range(NCH)
                        for kind in ("v", "k", "q")
                    ]
                    per_slot = 1
                    ready = lambda i, tqb: True
                else:
                    inj = [
                        lambda bb=bo, t8=t8: outproj(bb, t8)
                        for bo, t8s in ((b - 1, range(4, 16)), (b, range(12)))
                        for t8 in t8s
                    ]
                    per_slot = 2
                    # entry i >= 12 is outproj(b, i-12): its tqb block
                    # (i-12)//4 must have been normalized (tqb' < tqb)
                    ready = lambda i, tqb: i < 12 or (i - 12) < 4 * tqb
                ii = 0
                for tqb in range(1, NCH):
                    for p in range(8):
                        attn_p(b, tqb, p)
                        if p % 2 == 1:
                            for _ in range(per_slot):
                                if ii < len(inj) and ready(ii, tqb):
                                    inj[ii]()
                                    ii += 1
                    normalize(b, tqb)
                while ii < len(inj):
                    inj[ii]()
                    ii += 1

            # tail: last batch's final tqb output projection
            for tq8 in range(12, 16):
                outproj(B - 1, tq8)

    nc.compile()
    return nc


def _rope_tables():
    inv_freq = 1.0 / (BASE ** (np.arange(0, HD, 2, dtype=np.float64) / HD))
    t = np.arange(S, dtype=np.float64)
    freqs = np.outer(t, inv_freq)  # [S, 32]
    emb = np.concatenate([freqs, freqs], -1)  # [S, 64]
    cos = np.cos(emb).T.astype(np.float32)  # [64, S]
    sin = np.sin(emb).T.astype(np.float32)
    sin_signed = sin.copy()
    sin_signed[0:32] = -sin_signed[0:32]
    cosT = np.ascontiguousarray(np.tile(cos, (2, 1)))  # [128, S]
    sinTs = np.ascontiguousarray(np.tile(sin_signed, (2, 1)))
    return cosT, sinTs


def kernel(hidden_states, Wq, Wk, Wv, Wo):
    import ml_dtypes

    BF = ml_dtypes.bfloat16
    hidden_states = np.asarray(hidden_states, np.float32)
    Wq, Wk, Wv, Wo = (np.asarray(w, np.float32) for w in (Wq, Wk, Wv, Wo))

    if _nc_cache[0] is None:
        _nc_cache[0] = _build()
    nc = _nc_cache[0]

    hsT = np.ascontiguousarray(hidden_states.reshape(T, H).T).astype(BF)
    cosT, sinTs = _rope_tables()
    in_maps = []
    for c in range(NCORES):
        sl = slice(JC * c, JC * (c + 1))
        in_maps.append(
            {
                "hsT": hsT,
                "wqT": np.ascontiguousarray(Wq[sl, :].T).astype(BF),
                "wkT": np.ascontiguousarray(Wk[sl, :].T).astype(BF),
                "wvT": np.ascontiguousarray(Wv[sl, :].T).astype(BF),
                "woJI": np.ascontiguousarray(Wo[:, sl].T),
                "cosT": cosT,
                "sinTs": sinTs,
            }
        )

    from concourse.bass_utils import run_bass_kernel_spmd

    res = run_bass_kernel_spmd(nc, in_maps, core_ids=list(range(NCORES)))
    acc = np.zeros((T, H), np.float64)
    for c in range(NCORES):
        acc += res.results[c]["out"]
    return acc.astype(np.float32).reshape(B, S, H)


# revision 36
# speedup vs baseline: 1.3628x; 1.0928x over previous
"""Multi-head attention (B=2, S=2048, H=1024, NH=16, HD=64) on 8 trn2 cores.

Sharding: tensor-parallel over heads. Core c owns heads {2c, 2c+1}, i.e.
feature columns [128c, 128c+128) of q/k/v. Wq/Wk/Wv are column-sharded,
Wo row-sharded; each core computes a full-shape partial output and the
host sums the 8 partials (the row-parallel reduce) during unshard.

On-chip layout is feature-major ("transposed"): the host passes
hsT = hidden_states.T so both matmul operands of every projection have
the contraction dim on partitions. Attention works on scoresT[tk, tq];
softmax's normalizer comes from a ones-column augmented V matmul.

Schedule: QKV chains, attention p-blocks, next-batch chains and
prev-batch output-projection blocks are interleaved in emission order so
the ACT engine (exp) and Tensor engine stay busy from ~10us in.
QKV inputs (hsT, Wq/Wk/Wv) are bf16 (half the DMA, 1 cyc/row matmuls,
cheap LDWEIGHTS); attention q/k/v/et are fp16 (full PE rate; fp8 was
measured numerically too lossy for the 2e-2 gate: peaked softmax rows
expose elementwise quantization directly). Out-projection fp32r.
Normalizer: reciprocal_approx_fast + DRAM-broadcast; head-B ctx
accumulates on partitions 64:128 directly (ones col 0 of augmented V).
"""

import numpy as np

B, S, H, NH, HD = 2, 2048, 1024, 16, 64
NCORES = 8
JC = 128  # head-columns per core (2 heads x 64)
T = B * S  # 4096 tokens
TQB = 512  # tq block
NKT = S // 128  # 16 tk blocks per batch
NCH = S // TQB  # 4 token chunks per batch
BASE = 10000.0

_nc_cache = [None]

_LDW_OPT = False


def _patch_ldw_opt():
    from concourse import bass_utils as _bu

    if getattr(_bu, "_ldw_patched", False):
        return
    _orig = _bu.run_command

    def _patched(argv, **kw):
        argv = [
            a.replace("--enable-ldw-opt=false", "--enable-ldw-opt=true")
            if _LDW_OPT and isinstance(a, str)
            else a
            for a in argv
        ]
        return _orig(argv, **kw)

    _bu.run_command = _patched
    _bu._ldw_patched = True


def _build():
    _patch_ldw_opt()
    import concourse.tile as tile
    from concourse import bacc, mybir
    from concourse.masks import make_identity

    F32 = mybir.dt.float32
    F32R = mybir.dt.float32r
    F16 = mybir.dt.float16
    BF16 = mybir.dt.bfloat16
    EXP = mybir.ActivationFunctionType.Exp

    nc = bacc.Bacc("TRN2", target_bir_lowering=False, debug=False)

    hsT = nc.dram_tensor("hsT", [H, T], BF16, kind="ExternalInput").ap()
    wqT = nc.dram_tensor("wqT", [H, JC], BF16, kind="ExternalInput").ap()
    wkT = nc.dram_tensor("wkT", [H, JC], BF16, kind="ExternalInput").ap()
    wvT = nc.dram_tensor("wvT", [H, JC], BF16, kind="ExternalInput").ap()
    woJI = nc.dram_tensor("woJI", [JC, H], F32R, kind="ExternalInput").ap()
    cosT = nc.dram_tensor("cosT", [128, S], F32, kind="ExternalInput").ap()
    sinTs = nc.dram_tensor("sinTs", [128, S], F32, kind="ExternalInput").ap()
    out = nc.dram_tensor("out", [T, H], F32, kind="ExternalOutput").ap()

    with tile.TileContext(nc) as tc:
        with (
            tc.tile_pool(name="wts", bufs=1) as wts,
            tc.tile_pool(name="tabs", bufs=1) as tabs,
            tc.tile_pool(name="hst", bufs=3) as hst,
            tc.tile_pool(name="qkv", bufs=2) as qkvp,
            tc.tile_pool(name="rope", bufs=3) as ropep,
            tc.tile_pool(name="vaug", bufs=2) as vaugp,
            tc.tile_pool(name="expt", bufs=6) as exptp,
            tc.tile_pool(name="ctx", bufs=2) as ctxp,
            tc.tile_pool(name="nrm", bufs=2) as nrmp,
            tc.tile_pool(name="outs", bufs=6) as outsp,
            tc.tile_pool(name="scp", bufs=2, space="PSUM") as scp,
            tc.tile_pool(name="cxp", bufs=2, space="PSUM") as cxp,
            tc.tile_pool(name="ptp", bufs=2, space="PSUM") as ptp,
        ):
            # ---- persistent weights / tables; wv/wk/wq + first chunk first
            # so the first chains start ASAP ----
            wv_sb = wts.tile([128, 8, JC], BF16, tag="wv")
            nc.sync.dma_start(
                out=wv_sb[:], in_=wvT[:, :].rearrange("(k p) j -> p k j", p=128)
            )
            wk_sb = wts.tile([128, 8, JC], BF16, tag="wk")
            nc.sync.dma_start(
                out=wk_sb[:], in_=wkT[:, :].rearrange("(k p) j -> p k j", p=128)
            )
            wq_sb = wts.tile([128, 8, JC], BF16, tag="wq")
            nc.sync.dma_start(
                out=wq_sb[:], in_=wqT[:, :].rearrange("(k p) j -> p k j", p=128)
            )

            # ---- per-batch state, created lazily ----
            st = {}

            def batch_state(b):
                if b in st:
                    return st[b]
                qT = qkvp.tile([128, S], F16, tag="qT", name=f"qT{b}")
                kT = qkvp.tile([128, S], F16, tag="kT", bufs=1, name=f"kT{b}")
                vT = qkvp.tile([128, S], F16, tag="vT", bufs=1, name=f"vT{b}")
                kZA = qkvp.tile([128, S], F16, tag="kZA", name=f"kZA{b}")
                nc.vector.memset(kZA[64:128, :], 0.0)
                kZB = qkvp.tile([128, S], F16, tag="kZB", name=f"kZB{b}")
                nc.vector.memset(kZB[0:64, :], 0.0)
                # augmented V: head A ctx dims at cols 0:64, ones col 64
                #              head B ctx dims at cols 64:128, ones col 0
                vA = vaugp.tile([128, NKT, 128], F16, tag="vA", name=f"vA{b}")
                nc.vector.memset(vA[:, :, 65:128], 0.0)
                nc.vector.tensor_copy(vA[:, :, 64], onesc[:])
                vB = vaugp.tile([128, NKT, 128], F16, tag="vB", name=f"vB{b}")
                nc.vector.memset(vB[:, :, 1:64], 0.0)
                nc.vector.tensor_copy(vB[:, :, 0], onesc[:])
                ctxS = ctxp.tile([128, S], F32R, tag="cts", name=f"ctxS{b}")
                st[b] = dict(qT=qT, kT=kT, vT=vT, kZA=kZA, kZB=kZB, vA=vA,
                             vB=vB, ctxS=ctxS)
                return st[b]

            chunk_cache = {}

            def chunk(b, nchi):
                if (b, nchi) not in chunk_cache:
                    t0 = b * S + nchi * TQB
                    c = hst.tile([128, 8, TQB], BF16, tag="hst")
                    nc.sync.dma_start(
                        out=c[:],
                        in_=hsT[:, t0 : t0 + TQB].rearrange(
                            "(k p) t -> p k t", p=128
                        ),
                    )
                    chunk_cache[(b, nchi)] = c
                return chunk_cache[(b, nchi)]

            # prefetch the first token chunks ahead of the big table DMAs
            chunk(0, 0)
            chunk(0, 1)
            cos_sb = tabs.tile([128, S], F32, tag="cos")
            nc.sync.dma_start(out=cos_sb[:], in_=cosT[:, :])
            sin_sb = tabs.tile([128, S], F32, tag="sin")
            nc.sync.dma_start(out=sin_sb[:], in_=sinTs[:, :])
            wJ = wts.tile([128, H], F32R, tag="wj")
            nc.sync.dma_start(out=wJ[:], in_=woJI[:, :])
            ident = tabs.tile([128, 128], F16, tag="ident")
            make_identity(nc, ident[:])
            onesc = tabs.tile([128, NKT], F32, tag="ones")
            nc.vector.memset(onesc[:], 1.0)

            def chain(b, kind, nchi):
                sb = batch_state(b)
                c = chunk(b, nchi)
                sl = slice(nchi * TQB, (nchi + 1) * TQB)
                w_sb = {"q": wq_sb, "k": wk_sb, "v": wv_sb}[kind]
                pt = ptp.tile([128, TQB], F32, tag="pt")
                for k in range(8):
                    nc.tensor.matmul(
                        pt[:], w_sb[:, k, :], c[:, k, :],
                        start=(k == 0), stop=(k == 7),
                    )
                if kind == "v":
                    nc.vector.tensor_copy(sb["vT"][:, sl], pt[:])
                    for tkl in range(NCH):
                        tkb = NCH * nchi + tkl
                        pt2 = ptp.tile([128, 2 * TQB], F16, tag="pt")
                        nc.tensor.transpose(
                            pt2[:, 0:128],
                            sb["vT"][:, 128 * tkb : 128 * (tkb + 1)],
                            ident[:],
                        )
                        nc.vector.tensor_copy(
                            sb["vA"][:, tkb, 0:64], pt2[:, 0:64]
                        )
                        nc.vector.tensor_copy(
                            sb["vB"][:, tkb, 64:128], pt2[:, 64:128]
                        )
                    return
                # q/k: RoPE
                raw = ropep.tile([128, TQB], F32, tag="raw")
                nc.vector.tensor_copy(raw[:], pt[:])
                rot = ropep.tile([128, TQB], F32, tag="rot")
                for h0 in (0, 64):
                    nc.sync.dma_start(
                        out=rot[h0 : h0 + 32, :], in_=raw[h0 + 32 : h0 + 64, :]
                    )
                    nc.sync.dma_start(
                        out=rot[h0 + 32 : h0 + 64, :], in_=raw[h0 : h0 + 32, :]
                    )
                t1 = ropep.tile([128, TQB], F32, tag="t1")
                nc.vector.tensor_mul(t1[:], raw[:], cos_sb[:, sl])
                t2 = ropep.tile([128, TQB], F32, tag="t2")
                nc.vector.tensor_mul(t2[:], rot[:], sin_sb[:, sl])
                dstT = sb["qT"] if kind == "q" else sb["kT"]
                nc.vector.tensor_add(dstT[:, sl], t1[:], t2[:])
                if kind == "k":
                    nc.vector.tensor_copy(
                        sb["kZA"][0:64, sl], dstT[0:64, sl]
                    )
                    nc.vector.tensor_copy(
                        sb["kZB"][64:128, sl], dstT[64:128, sl]
                    )

            cx_live = {}

            def attn_p(b, tqb, p):
                sb = batch_state(b)
                qsl = slice(tqb * TQB, (tqb + 1) * TQB)
                if p == 0:
                    cx_live[(b, "A")] = cxp.tile([128, TQB], F32, tag="cx",
                                                 name=f"cxA{b}_{tqb}")
                    cx_live[(b, "B")] = cxp.tile([128, TQB], F32, tag="cx",
                                                 name=f"cxB{b}_{tqb}")
                cxA = cx_live[(b, "A")]
                cxB = cx_live[(b, "B")]
                scA = scp.tile([128, 2 * TQB], F32, tag="sc")
                scB = scp.tile([128, 2 * TQB], F32, tag="sc")
                for t in range(2):
                    tkb = 2 * p + t
                    ksl = slice(128 * tkb, 128 * (tkb + 1))
                    nc.tensor.matmul(
                        scA[:, t * TQB : (t + 1) * TQB],
                        sb["kZA"][:, ksl], sb["qT"][:, qsl],
                        start=True, stop=True,
                    )
                    nc.tensor.matmul(
                        scB[:, t * TQB : (t + 1) * TQB],
                        sb["kZB"][:, ksl], sb["qT"][:, qsl],
                        start=True, stop=True,
                    )
                etA = exptp.tile([128, 2 * TQB], F16, tag="et")
                nc.scalar.activation(etA[:], scA[:], EXP, scale=0.125)
                etB = exptp.tile([128, 2 * TQB], F16, tag="et")
                nc.scalar.activation(etB[:], scB[:], EXP, scale=0.125)
                for t in range(2):
                    tkb = 2 * p + t
                    stt, spp = tkb == 0, tkb == NKT - 1
                    tsl = slice(t * TQB, (t + 1) * TQB)
                    nc.tensor.matmul(
                        cxA[:, :], sb["vA"][:, tkb, :], etA[:, tsl],
                        start=stt, stop=spp,
                    )
                    nc.tensor.matmul(
                        cxB[:, :], sb["vB"][:, tkb, :], etB[:, tsl],
                        start=stt, stop=spp,
                    )

            def norm_half(src_row, dst, cx_rows, lo):
                zrow = nrmp.tile([1, TQB], F32, tag="zrow")
                nc.vector.tensor_copy(zrow[:], src_row)
                rz = nrmp.tile([1, TQB], F32, tag="rz")
                nc.vector.reciprocal_approx_fast(out=rz[:], in_=zrow[:])
                zr = nrmp.tile([128, TQB], F32, tag="zr")
                nc.gpsimd.partition_broadcast(zr[:], rz[:], channels=128)
                nc.vector.tensor_mul(dst, cx_rows, zr[lo : lo + 64, :])

            def normalize(b, tqb):
                sb = batch_state(b)
                qsl = slice(tqb * TQB, (tqb + 1) * TQB)
                cxA = cx_live.pop((b, "A"))
                cxB = cx_live.pop((b, "B"))
                norm_half(cxA[64:65, :], sb["ctxS"][0:64, qsl], cxA[0:64, :], 0)
                norm_half(cxB[0:1, :], sb["ctxS"][64:128, qsl],
                          cxB[64:128, :], 64)

            def outproj(b, tq8):
                sb = batch_state(b)
                csl = slice(128 * tq8, 128 * (tq8 + 1))
                for ich in range(2):
                    isl = slice(ich * 512, (ich + 1) * 512)
                    po = ptp.tile([128, TQB], F32, tag="pt")
                    nc.tensor.matmul(
                        po[:], sb["ctxS"][:, csl], wJ[:, isl],
                        start=True, stop=True,
                    )
                    ot = outsp.tile([128, TQB], F32, tag="ot")
                    nc.vector.tensor_copy(ot[:], po[:])
                    nc.sync.dma_start(
                        out=out[
                            b * S + 128 * tq8 : b * S + 128 * (tq8 + 1), isl
                        ],
                        in_=ot[:],
                    )

            # ---- master schedule ----
            for b in range(B):
                batch_state(b)
                if b == 0:
                    # tqb0 with this batch's chains interleaved
                    for nchi in range(NCH):
                        chain(b, "v", nchi)
                        chain(b, "k", nchi)
                        chain(b, "q", nchi)
                        attn_p(b, 0, 2 * nchi)
                        attn_p(b, 0, 2 * nchi + 1)
                else:
                    # chains already emitted during batch b-1; start draining
                    # the previous batch's output projection
                    for p in range(8):
                        attn_p(b, 0, p)
                        if p % 2 == 1:
                            outproj(b - 1, (p - 1) // 2)
                normalize(b, 0)
                # injection work for the p-odd slots of tqb 1..3:
                # non-final batch: the next batch's 12 chains (1 per slot).
                # final batch: remaining outproj blocks of batch b-1 and
                # this batch (2 per slot), gated on normalize readiness.
                if b + 1 < B:
                    inj = [
                        lambda b2=b + 1, kk=kind, nn=nchi: chain(b2, kk, nn)
                        for nchi in range(NCH)
                        for kind in ("v", "k", "q")
                    ]
                    per_slot = 1
                    ready = lambda i, tqb: True
                else:
                    inj = [
                        lambda bb=bo, t8=t8: outproj(bb, t8)
                        for bo, t8s in ((b - 1, range(4, 16)), (b, range(12)))
                        for t8 in t8s
                    ]
                    per_slot = 2
                    # entry i >= 12 is outproj(b, i-12): its tqb block
                    # (i-12)//4 must have been normalized (tqb' < tqb)
                    ready = lambda i, tqb: i < 12 or (i - 12) < 4 * tqb
                ii = 0
                for tqb in range(1, NCH):
                    for p in range(8):
                        attn_p(b, tqb, p)
                        if p % 2 == 1:
                            for _ in range(per_slot):
                                if ii < len(inj) and ready(ii, tqb):
                                    inj[ii]()
                                    ii += 1
                    normalize(b, tqb)
                while ii < len(inj):
                    inj[ii]()
                    ii += 1

            # tail: last batch's final tqb output projection
            for tq8 in range(12, 16):
                outproj(B - 1, tq8)

    nc.compile()
    return nc


def _rope_tables():
    inv_freq = 1.0 / (BASE ** (np.arange(0, HD, 2, dtype=np.float64) / HD))
    t = np.arange(S, dtype=np.float64)
    freqs = np.outer(t, inv_freq)  # [S, 32]
    emb = np.concatenate([freqs, freqs], -1)  # [S, 64]
    cos = np.cos(emb).T.astype(np.float32)  # [64, S]
    sin = np.sin(emb).T.astype(np.float32)
    sin_signed = sin.copy()
    sin_signed[0:32] = -sin_signed[0:32]
    cosT = np.ascontiguousarray(np.tile(cos, (2, 1)))  # [128, S]
    sinTs = np.ascontiguousarray(np.tile(sin_signed, (2, 1)))
    return cosT, sinTs


def kernel(hidden_states, Wq, Wk, Wv, Wo):
    import ml_dtypes

    BF = ml_dtypes.bfloat16
    hidden_states = np.asarray(hidden_states, np.float32)
    Wq, Wk, Wv, Wo = (np.asarray(w, np.float32) for w in (Wq, Wk, Wv, Wo))

    if _nc_cache[0] is None:
        _nc_cache[0] = _build()
    nc = _nc_cache[0]

    hsT = np.ascontiguousarray(hidden_states.reshape(T, H).T).astype(BF)
    cosT, sinTs = _rope_tables()
    in_maps = []
    for c in range(NCORES):
        sl = slice(JC * c, JC * (c + 1))
        in_maps.append(
            {
                "hsT": hsT,
                "wqT": np.ascontiguousarray(Wq[sl, :].T).astype(BF),
                "wkT": np.ascontiguousarray(Wk[sl, :].T).astype(BF),
                "wvT": np.ascontiguousarray(Wv[sl, :].T).astype(BF),
                "woJI": np.ascontiguousarray(Wo[:, sl].T),
                "cosT": cosT,
                "sinTs": sinTs,
            }
        )

    from concourse.bass_utils import run_bass_kernel_spmd

    res = run_bass_kernel_spmd(nc, in_maps, core_ids=list(range(NCORES)))
    acc = np.zeros((T, H), np.float64)
    for c in range(NCORES):
        acc += res.results[c]["out"]
    return acc.astype(np.float32).reshape(B, S, H)
